# revision 5
# baseline (speedup 1.0000x reference)
"""GCN (2-layer GraphConv + linear head) Trainium2 Bass kernel, 8 NeuronCores.

Strategy:
  - Symmetric norm D^-1/2 (A+I) D^-1/2 is separable: fold dis=1/sqrt(deg) into
    node features (u = (x@W) * dis), aggregate with plain sums, scale by dis[dst]
    via a diagonal moving operand in the PE matmul. Self loops become explicit
    edges.
  - Nodes are assigned to 8 cores (destination sharding). Per core, own nodes are
    sorted by in-degree so each 128-destination block has near-uniform degree.
  - The per-edge gather uses dma_gather (SWDGE, int16 indices). The global node
    table (101376 rows) is split into 4 windows of 25344 rows (= 2 core shards),
    one SWDGE queue per window. A host-side greedy balances each destination's
    edges across the 4 windows to minimize slot padding.
  - Aggregation: for each (block, window) slot-layer s, one matmul
    psum[F,128] += gathered[:,s,:].T @ diag(dis_block): contracts the 128 slot
    lanes, applies dis[dst], lands feature-major for the next layer's transform.
  - Layer-1 dense transform (x@W1) is computed replicated on every core (avoids
    an AllGather of u1); u2 requires one AllGather between the layers.
"""

import os
import sys

import numpy as np

sys.path.insert(0, "/opt/trn_rl_repo")

N = 100000
E = 3200000
IN_CH, HID, OUT = 256, 64, 32

NC = 8
NPC = 12544           # own nodes per core (98 blocks x 128)
SHARD = 12672         # table shard per core (NPC + 128 pad rows, mult of 128)
TROWS = NC * SHARD    # 101376 table rows
WIN = 2 * SHARD       # 25344 rows per gather window (int16-safe)
NWIN = 4
NB = NPC // 128       # 98 blocks
PADROW = NPC          # local index of the zero row inside each window

_CACHE = {}


def _install_ntff_shim():
    import types
    if "antenv.axon_hooks" in sys.modules:
        return
    mod = types.ModuleType("antenv.axon_hooks")
    _hook = [None]
    mod.set_axon_ntff_profile_hook = lambda h: _hook.__setitem__(0, h)
    mod.get_axon_ntff_profile_hook = lambda: _hook[0]
    sys.modules["antenv.axon_hooks"] = mod
    try:
        import antenv
        antenv.axon_hooks = mod
        from trn_agent_boot.trn_boot import _ntff_profile_via_ctypes
        mod.set_axon_ntff_profile_hook(_ntff_profile_via_ctypes("/opt/axon/libaxon_pjrt.so"))
    except Exception:
        pass


def _host_prep(edge_index):
    src = np.asarray(edge_index[0], dtype=np.int64)
    dst = np.asarray(edge_index[1], dtype=np.int64)
    deg = np.bincount(dst, minlength=N).astype(np.int64)

    # --- assign nodes to 4 window-pairs (greedy balance of per-dst window counts)
    order = np.argsort(-deg, kind="stable")
    # CSR over sources: out-edges of each node
    s_order = np.argsort(src, kind="stable")
    s_sorted = src[s_order]
    d_sorted = dst[s_order]
    indptr = np.searchsorted(s_sorted, np.arange(N + 1))

    quota = (deg + NWIN - 1) // NWIN + 1
    cnt = np.zeros((N, NWIN), np.int32)
    win_of = np.full(N, -1, np.int8)
    pair_load = np.zeros(NWIN, np.int64)
    pair_cap = 2 * NPC
    CH = 256
    for i0 in range(0, N, CH):
        nodes = order[i0:i0 + CH]
        starts = indptr[nodes]
        ends = indptr[nodes + 1]
        lens = ends - starts
        if lens.sum() == 0:
            scores = np.zeros((len(nodes), NWIN), np.float64)
        else:
            flat = np.concatenate([np.arange(s, e) for s, e in zip(starts, ends)])
            seg = np.repeat(np.arange(len(nodes)), lens)
            dd = d_sorted[flat]
            pressure = cnt[dd].astype(np.float64) + 1000.0 * (cnt[dd] + 1 > quota[dd, None])
            scores = np.zeros((len(nodes), NWIN), np.float64)
            np.add.at(scores, seg, pressure)
        scores += pair_load[None, :] * 1e-6
        scores[:, :] += np.where(pair_load >= pair_cap, 1e18, 0.0)[None, :]
        w = np.argmin(scores, axis=1).astype(np.int8)
        # capacity fixups (rare)
        for j, n in enumerate(nodes):
            wj = w[j]
            if pair_load[wj] >= pair_cap:
                wj = int(np.argmin(np.where(pair_load >= pair_cap, 1 << 60, pair_load)))
                w[j] = wj
            pair_load[wj] += 1
            win_of[n] = wj
        if lens.sum():
            np.add.at(cnt, (dd, w[seg]), 1)

    # --- owner core within pair, rank by degree inside core
    owner = np.full(N, -1, np.int16)
    rank = np.full(N, -1, np.int32)
    row_of = np.full(N, -1, np.int64)
    core_nodes = []
    for wpair in range(NWIN):
        members = order[win_of[order] == wpair]  # degree-desc within pair
        c0, c1 = 2 * wpair, 2 * wpair + 1
        a = members[0::2]
        b = members[1::2]
        for c, mem in ((c0, a), (c1, b)):
            owner[mem] = c
            rank[mem] = np.arange(len(mem))
            row_of[mem] = c * SHARD + rank[mem]
            core_nodes.append((c, mem))
    core_nodes = dict(core_nodes)

    # --- edges incl self-loops, in table coordinates
    src_all = np.concatenate([src, np.arange(N)])
    dst_all = np.concatenate([dst, np.arange(N)])
    srow = row_of[src_all]
    d_owner = owner[dst_all].astype(np.int64)
    d_rank = rank[dst_all].astype(np.int64)
    blk = d_rank >> 7
    lane = d_rank & 127
    w_e = srow // WIN
    local = (srow - w_e * WIN).astype(np.int64)

    # group edges by (core, block, window, lane); position within group = layer
    key = (((d_owner * NB + blk) * NWIN + w_e) * 128 + lane)
    ek = np.argsort(key, kind="stable")
    key_s = key[ek]
    local_s = local[ek]
    uniq, first_idx, counts = np.unique(key_s, return_index=True, return_counts=True)
    layer_s = np.arange(len(key_s)) - np.repeat(first_idx, counts)

    # per (core, block, window) max layer count over lanes -> global L over cores
    cbw = key_s // 128
    lane_s = key_s & 127
    maxcount = np.zeros(NC * NB * NWIN, np.int64)
    cnt_cbwl = np.zeros(NC * NB * NWIN * 128, np.int64)
    u2, c2 = np.unique(key_s, return_counts=True)
    cnt_cbwl[u2] = c2
    cl = cnt_cbwl.reshape(NC, NB, NWIN, 128)
    L = cl.max(axis=(0, 3))  # [NB, NWIN] global layer counts
    assert L.max() <= 29, f"gather too large for one SWDGE ring: L={L.max()}"

    # idx column offsets per (block, window): units of int16 columns (16 idx/col? no: 8L cols)
    cols = 8 * L  # [NB, NWIN]
    off = np.zeros((NB, NWIN), np.int64)
    run = 0
    for b in range(NB):
        for w in range(NWIN):
            off[b, w] = run
            run += int(cols[b, w])
    IDXCOLS = int(run)

    # build per-core idx arrays [128, IDXCOLS] int16, wrapped (j%16 -> partition band)
    slots_per = {}
    idx_flat = np.full((NC, NB, NWIN, 29, 128), PADROW, np.int32)  # layer-major
    idx_flat_view = idx_flat
    c_e = (cbw // (NB * NWIN)).astype(np.int64)
    b_e = (cbw // NWIN) % NB
    w_s = cbw % NWIN
    idx_flat_view[c_e, b_e, w_s, layer_s, lane_s] = local_s

    idx_arrays = []
    for c in range(NC):
        buf = np.empty((128, IDXCOLS), np.int16)
        for b in range(NB):
            for w in range(NWIN):
                Lw = int(L[b, w])
                if Lw == 0:
                    continue
                sl = idx_flat[c, b, w, :Lw, :].reshape(Lw * 128)  # slot j = layer*128+lane
                wrapped = sl.reshape(-1, 16).T.astype(np.int16)  # [16, 8*Lw]
                o = int(off[b, w])
                buf[:, o:o + 8 * Lw] = np.tile(wrapped, (8, 1))
        idx_arrays.append(buf)

    return dict(
        deg=deg, owner=owner, rank=rank, row_of=row_of, L=L, off=off,
        IDXCOLS=IDXCOLS, idx_arrays=idx_arrays, core_nodes=core_nodes,
    )


def _build_program(prep, has_b1, has_b2):
    import concourse.bacc as bacc
    import concourse.bass as bass
    import concourse.mybir as mybir
    import concourse.tile as tile

    L = prep["L"]
    off = prep["off"]
    IDXCOLS = prep["IDXCOLS"]
    NT = TROWS // 128  # 792 node tiles
    f32 = mybir.dt.float32

    nc = bacc.Bacc("TRN2", target_bir_lowering=False, debug=True,
                   num_swdge_queues=4, dynamic_dma_scratch_size=16384)

    xT = nc.dram_tensor("xT", [IN_CH, TROWS], f32, kind="ExternalInput")
    degT = nc.dram_tensor("degT", [TROWS], f32, kind="ExternalInput")
    dis_ownT = nc.dram_tensor("dis_own", [NPC], f32, kind="ExternalInput")
    W1T = nc.dram_tensor("W1", [IN_CH, HID], f32, kind="ExternalInput")
    W2T = nc.dram_tensor("W2", [HID, OUT], f32, kind="ExternalInput")
    WcT = nc.dram_tensor("Wc", [OUT, 1], f32, kind="ExternalInput")
    b1T_d = nc.dram_tensor("b1", [HID], f32, kind="ExternalInput")
    b2T_d = nc.dram_tensor("b2", [OUT], f32, kind="ExternalInput")
    bcT_d = nc.dram_tensor("bc", [128], f32, kind="ExternalInput")
    identT = nc.dram_tensor("ident", [128, 128], f32, kind="ExternalInput")
    idxT = nc.dram_tensor("idx", [128, IDXCOLS], mybir.dt.int16, kind="ExternalInput")
    outT = nc.dram_tensor("out", [NPC], f32, kind="ExternalOutput")

    bf16 = mybir.dt.bfloat16
    TCOLS = 128
    u1_tab = nc.dram_tensor("u1_tab", [TROWS, TCOLS], bf16)
    u2_shard = nc.dram_tensor("u2_shard", [SHARD, TCOLS], bf16)
    u2_full = nc.dram_tensor("u2_full", [TROWS, TCOLS], bf16, addr_space="Shared")

    with tile.TileContext(nc) as tc:
        with tc.tile_pool(name="const", bufs=1) as cpool:
            ident = cpool.tile([128, 128], f32)
            nc.sync.dma_start(out=ident[:], in_=identT[:])
            W1s = cpool.tile([128, 2, HID], f32)
            nc.sync.dma_start(out=W1s[:], in_=W1T.ap().rearrange("(k p) h -> p k h", p=128))
            W2s = cpool.tile([HID, OUT], f32)
            nc.sync.dma_start(out=W2s[:], in_=W2T[:])
            Wcs = cpool.tile([OUT, 1], f32)
            nc.sync.dma_start(out=Wcs[:], in_=WcT[:])
            b1s = cpool.tile([HID, 1], f32)
            nc.sync.dma_start(out=b1s[:], in_=b1T_d.ap().rearrange("(h one) -> h one", one=1))
            b2s = cpool.tile([OUT, 1], f32)
            nc.sync.dma_start(out=b2s[:], in_=b2T_d.ap().rearrange("(h one) -> h one", one=1))
            bcs = cpool.tile([128, 1], f32)
            nc.sync.dma_start(out=bcs[:], in_=bcT_d.ap().rearrange("(h one) -> h one", one=1))
            dis_sb = cpool.tile([128, NT], f32)
            nc.sync.dma_start(out=dis_sb[:], in_=degT.ap().rearrange("(t p) -> p t", p=128))
            # dis = 1/sqrt(deg): sqrt on ACT, reciprocal on DVE
            nc.scalar.activation(dis_sb[:], dis_sb[:], mybir.ActivationFunctionType.Sqrt)
            nc.vector.reciprocal(dis_sb[:], dis_sb[:])
            dis_own = cpool.tile([128, NB], f32)
            nc.sync.dma_start(out=dis_own[:], in_=dis_ownT.ap().rearrange("(b p) -> p b", p=128))
            zrow = cpool.tile([128, 128], bf16)
            nc.vector.memset(zrow[:], 0.0)
            # zero pad region of u2_shard (rows NPC..SHARD) so pad gathers read zeros
            nc.sync.dma_start(out=u2_shard[NPC:NPC + 128, :], in_=zrow[:])
            out_sb = cpool.tile([128, NB], f32)

            # ---------- phase A: u1 table (replicated dense transform) ----------
            with tc.tile_pool(name="pa", bufs=3) as pa, \
                 tc.tile_pool(name="pa_ps", bufs=2, space="PSUM") as pa_ps:
                for t in range(NT):
                    xt = pa.tile([128, 2, 128], f32, tag="xt")
                    eng = nc.sync if (t & 1) == 0 else nc.scalar
                    eng.dma_start(
                        out=xt[:],
                        in_=xT[:, t * 128:(t + 1) * 128].rearrange("(k p) n -> p k n", p=128),
                    )
                    ps = pa_ps.tile([128, HID], f32)
                    nc.tensor.matmul(ps[:], lhsT=xt[:, 0, :], rhs=W1s[:, 0, :], start=True, stop=False)
                    nc.tensor.matmul(ps[:], lhsT=xt[:, 1, :], rhs=W1s[:, 1, :], start=False, stop=True)
                    u1t = pa.tile([128, HID], bf16, tag="u1t")
                    nc.vector.tensor_scalar(u1t[:], ps[:], dis_sb[:, t:t + 1], None, mybir.AluOpType.mult)
                    eng2 = nc.scalar if (t & 1) == 0 else nc.sync
                    eng2.dma_start(out=u1_tab[t * 128:(t + 1) * 128, 0:HID], in_=u1t[:])

            # ---------- aggregation layers ----------
            def agg_layer(table, feat, W_next, b_tile, has_b, layer):
                # per block: gathers from 4 windows -> psum[feat,128] accum -> finish
                with tc.tile_pool(name=f"gi{layer}", bufs=3) as ip, \
                     tc.tile_pool(name=f"gg{layer}", bufs=8) as gp, \
                     tc.tile_pool(name=f"fin{layer}", bufs=3) as fp, \
                     tc.tile_pool(name=f"ps{layer}", bufs=3, space="PSUM") as pp, \
                     tc.tile_pool(name=f"ps2{layer}", bufs=2, space="PSUM") as pp2:
                    pending = None

                    def finish_block(b, acc):
                        if has_b:
                            s1 = fp.tile([feat, 128], f32, tag="s1")
                            nc.vector.tensor_scalar(s1[:], acc[:], b_tile[:, 0:1], None,
                                                    mybir.AluOpType.add)
                            base = s1
                        else:
                            base = acc
                        m = fp.tile([feat, 128], f32, tag="m")
                        nc.vector.tensor_scalar(m[:], base[:], 0.0, None, mybir.AluOpType.min)
                        e = fp.tile([feat, 128], f32, tag="e")
                        nc.scalar.activation(e[:], m[:], mybir.ActivationFunctionType.Exp)
                        r = fp.tile([feat, 128], f32, tag="r")
                        nc.vector.tensor_scalar(r[:], base[:], 0.0, -1.0,
                                                mybir.AluOpType.max, mybir.AluOpType.add)
                        h = fp.tile([feat, 128], f32, tag="h")
                        nc.vector.tensor_tensor(out=h[:], in0=e[:], in1=r[:], op=mybir.AluOpType.add)
                        if layer == 1:
                            t2 = pp2.tile([128, OUT], f32, tag="t2")
                            nc.tensor.matmul(t2[:], lhsT=h[:], rhs=W_next[:], start=True, stop=True)
                            u2t = fp.tile([128, OUT], bf16, tag="u2t")
                            nc.vector.tensor_scalar(u2t[:], t2[:], dis_own[:, b:b + 1], None,
                                                    mybir.AluOpType.mult)
                            nc.scalar.dma_start(out=u2_shard[b * 128:(b + 1) * 128, 0:OUT], in_=u2t[:])
                        else:
                            hp = pp2.tile([128, 1], f32, tag="hp")
                            nc.tensor.matmul(hp[:], lhsT=h[:], rhs=Wcs[:], start=True, stop=True)
                            nc.vector.tensor_scalar(out_sb[:, b:b + 1], hp[:], bcs[:, 0:1], None,
                                                    mybir.AluOpType.add)

                    for b in range(NB):
                        diag = fp.tile([128, 128], bf16, tag="diag")
                        nc.vector.tensor_scalar(diag[:], ident[:], dis_own[:, b:b + 1], None,
                                                mybir.AluOpType.mult)
                        acc = pp.tile([feat, 128], f32, tag="acc")
                        nmm = int(L[b].sum())
                        k = 0
                        blk_cols = int(8 * L[b].sum())
                        it = ip.tile([128, blk_cols], mybir.dt.int16, tag="it")
                        nc.scalar.dma_start(out=it[:], in_=idxT[:, int(off[b, 0]):int(off[b, 0]) + blk_cols])
                        icol = 0
                        for w in range(NWIN):
                            Lw = int(L[b, w])
                            if Lw == 0:
                                continue
                            g = gp.tile([128, Lw, TCOLS], bf16, tag=f"g{w}")
                            nc.gpsimd.dma_gather(
                                out_ap=g[:],
                                in_ap=table[w * WIN:(w + 1) * WIN, :],
                                idxs_ap=it[:, icol:icol + 8 * Lw],
                                num_idxs=128 * Lw,
                                num_idxs_reg=128 * Lw,
                                elem_size=TCOLS,
                                single_packet=False,
                                queue_num=w,
                            )
                            icol += 8 * Lw
                            for s in range(Lw):
                                nc.tensor.matmul(acc[:], lhsT=g[:, s, 0:feat], rhs=diag[:],
                                                 start=(k == 0), stop=(k == nmm - 1))
                                k += 1
                        if pending is not None:
                            finish_block(*pending)
                        pending = (b, acc)
                    finish_block(*pending)
                    return None

            agg_layer(u1_tab, HID, W2s, b1s, has_b1, layer=1)

            # ---------- AllGather u2 ----------
            nc.gpsimd.collective_compute(
                "AllGather",
                mybir.AluOpType.bypass,
                replica_groups=[list(range(NC))],
                ins=[u2_shard[:]],
                outs=[u2_full[:]],
            )

            agg_layer(u2_full, OUT, None, b2s, has_b2, layer=2)

            nc.sync.dma_start(out=outT.ap().rearrange("(b p) -> p b", p=128), in_=out_sb[:])

    nc.compile()
    return nc


def kernel(x, edge_index, W1, b1, W2, b2, Wc, bc):
    from concourse.bass_utils import run_bass_kernel_spmd

    x = np.asarray(x, dtype=np.float32)
    edge_index = np.asarray(edge_index)
    W1 = np.asarray(W1, np.float32); b1 = np.asarray(b1, np.float32)
    W2 = np.asarray(W2, np.float32); b2 = np.asarray(b2, np.float32)
    Wc = np.asarray(Wc, np.float32); bc = np.asarray(bc, np.float32)

    key = hash(edge_index.tobytes())
    if key not in _CACHE:
        prep = _host_prep(edge_index)
        prog = _build_program(prep, bool(np.any(b1)), bool(np.any(b2)))
        _CACHE[key] = (prep, prog)
    prep, prog = _CACHE[key]

    row_of = prep["row_of"]
    deg = prep["deg"]

    xT = np.zeros((IN_CH, TROWS), np.float32)
    xT[:, row_of] = x.T
    degT = np.ones(TROWS, np.float32)
    degT[row_of] = (deg + 1).astype(np.float32)

    dis_full = 1.0 / np.sqrt((deg + 1).astype(np.float32))

    in_maps = []
    for c in range(NC):
        dis_own = np.ones(NPC, np.float32)
        mem = prep["core_nodes"][c]
        dis_own[:len(mem)] = dis_full[mem]
        in_maps.append(dict(
            xT=xT, degT=degT, dis_own=dis_own,
            W1=W1, W2=W2, Wc=Wc.reshape(OUT, 1),
            b1=b1, b2=b2, bc=np.full(128, float(bc.reshape(-1)[0]), np.float32),
            ident=np.eye(128, dtype=np.float32),
            idx=prep["idx_arrays"][c],
        ))

    trace = os.environ.get("GCN_TRACE", "0") == "1"
    if trace:
        _install_ntff_shim()
        import concourse.bass_utils as bu
        bu.upload_artifacts = lambda d: "local://" + str(d)
    res = run_bass_kernel_spmd(prog, in_maps, list(range(NC)), trace=trace)
    if trace:
        kernel.last_exec_time_ns = res.exec_time_ns

    out = np.zeros(N, np.float32)
    for c in range(NC):
        mem = prep["core_nodes"][c]
        out[mem] = res.results[c]["out"][:len(mem)]
    return out


kernel.last_exec_time_ns = None


# revision 7
# speedup vs baseline: 1.0556x; 1.0556x over previous
"""GCN (2-layer GraphConv + linear head) Trainium2 Bass kernel, 8 NeuronCores.

Strategy:
  - Symmetric norm D^-1/2 (A+I) D^-1/2 is separable: fold dis=1/sqrt(deg) into
    node features (u = (x@W) * dis), aggregate with plain sums, scale by dis[dst]
    via a diagonal moving operand in the PE matmul. Self loops become explicit
    edges.
  - Nodes are assigned to 8 cores (destination sharding). Per core, own nodes are
    sorted by in-degree so each 128-destination block has near-uniform degree.
  - The per-edge gather uses dma_gather (SWDGE, int16 indices). The global node
    table (101376 rows) is split into 4 windows of 25344 rows (= 2 core shards),
    one SWDGE queue per window. A host-side greedy balances each destination's
    edges across the 4 windows to minimize slot padding.
  - Aggregation: for each (block, window) slot-layer s, one matmul
    psum[F,128] += gathered[:,s,:].T @ diag(dis_block): contracts the 128 slot
    lanes, applies dis[dst], lands feature-major for the next layer's transform.
  - Layer-1 dense transform (x@W1) is computed replicated on every core (avoids
    an AllGather of u1); u2 requires one AllGather between the layers.
"""

import os
import sys

import numpy as np

sys.path.insert(0, "/opt/trn_rl_repo")

N = 100000
E = 3200000
IN_CH, HID, OUT = 256, 64, 32

NC = 8
NPC = 12544           # own nodes per core (98 blocks x 128)
SHARD = 12672         # table shard per core (NPC + 128 pad rows, mult of 128)
TROWS = NC * SHARD    # 101376 table rows
WIN = 2 * SHARD       # 25344 rows per gather window (int16-safe)
NWIN = 4
NB = NPC // 128       # 98 blocks
PADROW = NPC          # local index of the zero row inside each window

_CACHE = {}


def _install_ntff_shim():
    import types
    if "antenv.axon_hooks" in sys.modules:
        return
    mod = types.ModuleType("antenv.axon_hooks")
    _hook = [None]
    mod.set_axon_ntff_profile_hook = lambda h: _hook.__setitem__(0, h)
    mod.get_axon_ntff_profile_hook = lambda: _hook[0]
    sys.modules["antenv.axon_hooks"] = mod
    try:
        import antenv
        antenv.axon_hooks = mod
        from trn_agent_boot.trn_boot import _ntff_profile_via_ctypes
        mod.set_axon_ntff_profile_hook(_ntff_profile_via_ctypes("/opt/axon/libaxon_pjrt.so"))
    except Exception:
        pass


def _host_prep(edge_index):
    src = np.asarray(edge_index[0], dtype=np.int64)
    dst = np.asarray(edge_index[1], dtype=np.int64)
    deg = np.bincount(dst, minlength=N).astype(np.int64)

    # --- assign nodes to 4 window-pairs (greedy balance of per-dst window counts)
    order = np.argsort(-deg, kind="stable")
    # CSR over sources: out-edges of each node
    s_order = np.argsort(src, kind="stable")
    s_sorted = src[s_order]
    d_sorted = dst[s_order]
    indptr = np.searchsorted(s_sorted, np.arange(N + 1))

    quota = (deg + NWIN - 1) // NWIN + 1
    cnt = np.zeros((N, NWIN), np.int32)
    win_of = np.full(N, -1, np.int8)
    pair_load = np.zeros(NWIN, np.int64)
    pair_cap = 2 * NPC
    CH = 256
    for i0 in range(0, N, CH):
        nodes = order[i0:i0 + CH]
        starts = indptr[nodes]
        ends = indptr[nodes + 1]
        lens = ends - starts
        if lens.sum() == 0:
            scores = np.zeros((len(nodes), NWIN), np.float64)
        else:
            flat = np.concatenate([np.arange(s, e) for s, e in zip(starts, ends)])
            seg = np.repeat(np.arange(len(nodes)), lens)
            dd = d_sorted[flat]
            pressure = cnt[dd].astype(np.float64) + 1000.0 * (cnt[dd] + 1 > quota[dd, None])
            scores = np.zeros((len(nodes), NWIN), np.float64)
            np.add.at(scores, seg, pressure)
        scores += pair_load[None, :] * 1e-6
        scores[:, :] += np.where(pair_load >= pair_cap, 1e18, 0.0)[None, :]
        w = np.argmin(scores, axis=1).astype(np.int8)
        # capacity fixups (rare)
        for j, n in enumerate(nodes):
            wj = w[j]
            if pair_load[wj] >= pair_cap:
                wj = int(np.argmin(np.where(pair_load >= pair_cap, 1 << 60, pair_load)))
                w[j] = wj
            pair_load[wj] += 1
            win_of[n] = wj
        if lens.sum():
            np.add.at(cnt, (dd, w[seg]), 1)

    # --- owner core within pair, rank by degree inside core
    owner = np.full(N, -1, np.int16)
    rank = np.full(N, -1, np.int32)
    row_of = np.full(N, -1, np.int64)
    core_nodes = []
    for wpair in range(NWIN):
        members = order[win_of[order] == wpair]  # degree-desc within pair
        c0, c1 = 2 * wpair, 2 * wpair + 1
        a = members[0::2]
        b = members[1::2]
        for c, mem in ((c0, a), (c1, b)):
            owner[mem] = c
            rank[mem] = np.arange(len(mem))
            row_of[mem] = c * SHARD + rank[mem]
            core_nodes.append((c, mem))
    core_nodes = dict(core_nodes)

    # --- edges incl self-loops, in table coordinates
    src_all = np.concatenate([src, np.arange(N)])
    dst_all = np.concatenate([dst, np.arange(N)])
    srow = row_of[src_all]
    d_owner = owner[dst_all].astype(np.int64)
    d_rank = rank[dst_all].astype(np.int64)
    blk = d_rank >> 7
    lane = d_rank & 127
    w_e = srow // WIN
    local = (srow - w_e * WIN).astype(np.int64)

    # group edges by (core, block, window, lane); position within group = layer
    key = (((d_owner * NB + blk) * NWIN + w_e) * 128 + lane)
    ek = np.argsort(key, kind="stable")
    key_s = key[ek]
    local_s = local[ek]
    uniq, first_idx, counts = np.unique(key_s, return_index=True, return_counts=True)
    layer_s = np.arange(len(key_s)) - np.repeat(first_idx, counts)

    # per (core, block, window) max layer count over lanes -> global L over cores
    cbw = key_s // 128
    lane_s = key_s & 127
    maxcount = np.zeros(NC * NB * NWIN, np.int64)
    cnt_cbwl = np.zeros(NC * NB * NWIN * 128, np.int64)
    u2, c2 = np.unique(key_s, return_counts=True)
    cnt_cbwl[u2] = c2
    cl = cnt_cbwl.reshape(NC, NB, NWIN, 128)
    L = cl.max(axis=(0, 3))  # [NB, NWIN] global layer counts
    assert L.max() <= 29, f"gather too large for one SWDGE ring: L={L.max()}"

    # idx column offsets per (block, window): units of int16 columns (16 idx/col? no: 8L cols)
    cols = 8 * L  # [NB, NWIN]
    off = np.zeros((NB, NWIN), np.int64)
    run = 0
    for b in range(NB):
        for w in range(NWIN):
            off[b, w] = run
            run += int(cols[b, w])
    IDXCOLS = int(run)

    # build per-core idx arrays [128, IDXCOLS] int16, wrapped (j%16 -> partition band)
    slots_per = {}
    idx_flat = np.full((NC, NB, NWIN, 29, 128), PADROW, np.int32)  # layer-major
    idx_flat_view = idx_flat
    c_e = (cbw // (NB * NWIN)).astype(np.int64)
    b_e = (cbw // NWIN) % NB
    w_s = cbw % NWIN
    idx_flat_view[c_e, b_e, w_s, layer_s, lane_s] = local_s

    idx_arrays = []
    for c in range(NC):
        buf = np.empty((128, IDXCOLS), np.int16)
        for b in range(NB):
            for w in range(NWIN):
                Lw = int(L[b, w])
                if Lw == 0:
                    continue
                sl = idx_flat[c, b, w, :Lw, :].reshape(Lw * 128)  # slot j = layer*128+lane
                wrapped = sl.reshape(-1, 16).T.astype(np.int16)  # [16, 8*Lw]
                o = int(off[b, w])
                buf[:, o:o + 8 * Lw] = np.tile(wrapped, (8, 1))
        idx_arrays.append(buf)

    return dict(
        deg=deg, owner=owner, rank=rank, row_of=row_of, L=L, off=off,
        IDXCOLS=IDXCOLS, idx_arrays=idx_arrays, core_nodes=core_nodes,
    )


def _build_program(prep, has_b1, has_b2):
    import concourse.bacc as bacc
    import concourse.bass as bass
    import concourse.mybir as mybir
    import concourse.tile as tile

    L = prep["L"]
    off = prep["off"]
    IDXCOLS = prep["IDXCOLS"]
    NT = TROWS // 128  # 792 node tiles
    f32 = mybir.dt.float32

    nc = bacc.Bacc("TRN2", target_bir_lowering=False, debug=True,
                   num_swdge_queues=4, dynamic_dma_scratch_size=16384)

    xT = nc.dram_tensor("xT", [IN_CH, TROWS], f32, kind="ExternalInput")
    degT = nc.dram_tensor("degT", [TROWS], f32, kind="ExternalInput")
    dis_ownT = nc.dram_tensor("dis_own", [NPC], f32, kind="ExternalInput")
    W1T = nc.dram_tensor("W1", [IN_CH, HID], f32, kind="ExternalInput")
    W2T = nc.dram_tensor("W2", [HID, OUT], f32, kind="ExternalInput")
    WcT = nc.dram_tensor("Wc", [OUT, 1], f32, kind="ExternalInput")
    b1T_d = nc.dram_tensor("b1", [HID], f32, kind="ExternalInput")
    b2T_d = nc.dram_tensor("b2", [OUT], f32, kind="ExternalInput")
    bcT_d = nc.dram_tensor("bc", [128], f32, kind="ExternalInput")
    identT = nc.dram_tensor("ident", [128, 128], f32, kind="ExternalInput")
    idxT = nc.dram_tensor("idx", [128, IDXCOLS], mybir.dt.int16, kind="ExternalInput")
    outT = nc.dram_tensor("out", [NPC], f32, kind="ExternalOutput")

    u1_tab = nc.dram_tensor("u1_tab", [TROWS, HID], f32)
    u2_shard = nc.dram_tensor("u2_shard", [SHARD, HID], f32)
    u2_full = nc.dram_tensor("u2_full", [TROWS, HID], f32, addr_space="Shared")

    with tile.TileContext(nc) as tc:
        with tc.tile_pool(name="const", bufs=1) as cpool:
            ident = cpool.tile([128, 128], f32)
            nc.sync.dma_start(out=ident[:], in_=identT[:])
            W1s = cpool.tile([128, 2, HID], f32)
            nc.sync.dma_start(out=W1s[:], in_=W1T.ap().rearrange("(k p) h -> p k h", p=128))
            W2s = cpool.tile([HID, OUT], f32)
            nc.sync.dma_start(out=W2s[:], in_=W2T[:])
            Wcs = cpool.tile([OUT, 1], f32)
            nc.sync.dma_start(out=Wcs[:], in_=WcT[:])
            b1s = cpool.tile([HID, 1], f32)
            nc.sync.dma_start(out=b1s[:], in_=b1T_d.ap().rearrange("(h one) -> h one", one=1))
            b2s = cpool.tile([OUT, 1], f32)
            nc.sync.dma_start(out=b2s[:], in_=b2T_d.ap().rearrange("(h one) -> h one", one=1))
            bcs = cpool.tile([128, 1], f32)
            nc.sync.dma_start(out=bcs[:], in_=bcT_d.ap().rearrange("(h one) -> h one", one=1))
            dis_sb = cpool.tile([128, NT], f32)
            nc.sync.dma_start(out=dis_sb[:], in_=degT.ap().rearrange("(t p) -> p t", p=128))
            # dis = 1/sqrt(deg): sqrt on ACT, reciprocal on DVE
            nc.scalar.activation(dis_sb[:], dis_sb[:], mybir.ActivationFunctionType.Sqrt)
            nc.vector.reciprocal(dis_sb[:], dis_sb[:])
            dis_own = cpool.tile([128, NB], f32)
            nc.sync.dma_start(out=dis_own[:], in_=dis_ownT.ap().rearrange("(b p) -> p b", p=128))
            zrow = cpool.tile([128, HID], f32)
            nc.vector.memset(zrow[:], 0.0)
            # zero pad region of u2_shard (rows NPC..SHARD) so pad gathers read zeros
            nc.sync.dma_start(out=u2_shard[NPC:NPC + 128, :], in_=zrow[:])
            out_sb = cpool.tile([128, NB], f32)

            # ---------- phase A: u1 table (replicated dense transform) ----------
            with tc.tile_pool(name="pa", bufs=3) as pa, \
                 tc.tile_pool(name="pa_ps", bufs=2, space="PSUM") as pa_ps:
                for t0 in range(0, NT, 2):
                    xt = pa.tile([128, 2, 256], f32, tag="xt")
                    eng = nc.sync if (t0 & 2) == 0 else nc.scalar
                    eng.dma_start(
                        out=xt[:],
                        in_=xT[:, t0 * 128:(t0 + 2) * 128].rearrange("(k p) n -> p k n", p=128),
                    )
                    u1t = pa.tile([128, 2, HID], f32, tag="u1t")
                    for j in range(2):
                        t = t0 + j
                        ps = pa_ps.tile([128, HID], f32)
                        nc.tensor.matmul(ps[:], lhsT=xt[:, 0, j * 128:(j + 1) * 128],
                                         rhs=W1s[:, 0, :], start=True, stop=False)
                        nc.tensor.matmul(ps[:], lhsT=xt[:, 1, j * 128:(j + 1) * 128],
                                         rhs=W1s[:, 1, :], start=False, stop=True)
                        nc.vector.tensor_scalar(u1t[:, j, :], ps[:], dis_sb[:, t:t + 1], None,
                                                mybir.AluOpType.mult)
                    eng2 = nc.scalar if (t0 & 2) == 0 else nc.sync
                    eng2.dma_start(
                        out=u1_tab[t0 * 128:(t0 + 2) * 128, :].rearrange("(j p) h -> p j h", p=128),
                        in_=u1t[:],
                    )

            # ---------- aggregation layers ----------
            def agg_layer(table, feat, W_next, b_tile, has_b, layer):
                # per block: gathers from 4 windows -> psum[feat,128] accum -> finish
                with tc.tile_pool(name=f"gi{layer}", bufs=3) as ip, \
                     tc.tile_pool(name=f"gg{layer}", bufs=8) as gp, \
                     tc.tile_pool(name=f"fin{layer}", bufs=3) as fp, \
                     tc.tile_pool(name=f"ps{layer}", bufs=3, space="PSUM") as pp, \
                     tc.tile_pool(name=f"ps2{layer}", bufs=2, space="PSUM") as pp2:
                    pending = None

                    def finish_block(b, acc):
                        if has_b:
                            s1 = fp.tile([feat, 128], f32, tag="s1")
                            nc.vector.tensor_scalar(s1[:], acc[:], b_tile[:, 0:1], None,
                                                    mybir.AluOpType.add)
                            base = s1
                        else:
                            base = acc
                        m = fp.tile([feat, 128], f32, tag="m")
                        nc.vector.tensor_scalar(m[:], base[:], 0.0, None, mybir.AluOpType.min)
                        e = fp.tile([feat, 128], f32, tag="e")
                        nc.scalar.activation(e[:], m[:], mybir.ActivationFunctionType.Exp)
                        r = fp.tile([feat, 128], f32, tag="r")
                        nc.vector.tensor_scalar(r[:], base[:], 0.0, -1.0,
                                                mybir.AluOpType.max, mybir.AluOpType.add)
                        h = fp.tile([feat, 128], f32, tag="h")
                        nc.vector.tensor_tensor(out=h[:], in0=e[:], in1=r[:], op=mybir.AluOpType.add)
                        if layer == 1:
                            t2 = pp2.tile([128, OUT], f32, tag="t2")
                            nc.tensor.matmul(t2[:], lhsT=h[:], rhs=W_next[:], start=True, stop=True)
                            u2t = fp.tile([128, OUT], f32, tag="u2t")
                            nc.vector.tensor_scalar(u2t[:], t2[:], dis_own[:, b:b + 1], None,
                                                    mybir.AluOpType.mult)
                            nc.scalar.dma_start(out=u2_shard[b * 128:(b + 1) * 128, 0:OUT], in_=u2t[:])
                        else:
                            hp = pp2.tile([128, 1], f32, tag="hp")
                            nc.tensor.matmul(hp[:], lhsT=h[:], rhs=Wcs[:], start=True, stop=True)
                            nc.vector.tensor_scalar(out_sb[:, b:b + 1], hp[:], bcs[:, 0:1], None,
                                                    mybir.AluOpType.add)

                    for b in range(NB):
                        diag = fp.tile([128, 128], f32, tag="diag")
                        nc.vector.tensor_scalar(diag[:], ident[:], dis_own[:, b:b + 1], None,
                                                mybir.AluOpType.mult)
                        acc = pp.tile([feat, 128], f32, tag="acc")
                        nmm = int(L[b].sum())
                        k = 0
                        blk_cols = int(8 * L[b].sum())
                        it = ip.tile([128, blk_cols], mybir.dt.int16, tag="it")
                        nc.scalar.dma_start(out=it[:], in_=idxT[:, int(off[b, 0]):int(off[b, 0]) + blk_cols])
                        icol = 0
                        for w in range(NWIN):
                            Lw = int(L[b, w])
                            if Lw == 0:
                                continue
                            g = gp.tile([128, Lw, HID], f32, tag=f"g{w}")
                            nc.gpsimd.dma_gather(
                                out_ap=g[:],
                                in_ap=table[w * WIN:(w + 1) * WIN, :],
                                idxs_ap=it[:, icol:icol + 8 * Lw],
                                num_idxs=128 * Lw,
                                num_idxs_reg=128 * Lw,
                                elem_size=HID,
                                single_packet=False,
                                queue_num=w,
                            )
                            icol += 8 * Lw
                            for s in range(Lw):
                                nc.tensor.matmul(acc[:], lhsT=g[:, s, 0:feat], rhs=diag[:],
                                                 start=(k == 0), stop=(k == nmm - 1))
                                k += 1
                        if pending is not None:
                            finish_block(*pending)
                        pending = (b, acc)
                    finish_block(*pending)
                    return None

            agg_layer(u1_tab, HID, W2s, b1s, has_b1, layer=1)

            # ---------- AllGather u2 ----------
            nc.gpsimd.collective_compute(
                "AllGather",
                mybir.AluOpType.bypass,
                replica_groups=[list(range(NC))],
                ins=[u2_shard[:]],
                outs=[u2_full[:]],
            )

            agg_layer(u2_full, OUT, None, b2s, has_b2, layer=2)

            nc.sync.dma_start(out=outT.ap().rearrange("(b p) -> p b", p=128), in_=out_sb[:])

    nc.compile()
    return nc


def kernel(x, edge_index, W1, b1, W2, b2, Wc, bc):
    from concourse.bass_utils import run_bass_kernel_spmd

    x = np.asarray(x, dtype=np.float32)
    edge_index = np.asarray(edge_index)
    W1 = np.asarray(W1, np.float32); b1 = np.asarray(b1, np.float32)
    W2 = np.asarray(W2, np.float32); b2 = np.asarray(b2, np.float32)
    Wc = np.asarray(Wc, np.float32); bc = np.asarray(bc, np.float32)

    key = hash(edge_index.tobytes())
    if key not in _CACHE:
        prep = _host_prep(edge_index)
        prog = _build_program(prep, bool(np.any(b1)), bool(np.any(b2)))
        _CACHE[key] = (prep, prog)
    prep, prog = _CACHE[key]

    row_of = prep["row_of"]
    deg = prep["deg"]

    xT = np.zeros((IN_CH, TROWS), np.float32)
    xT[:, row_of] = x.T
    degT = np.ones(TROWS, np.float32)
    degT[row_of] = (deg + 1).astype(np.float32)

    dis_full = 1.0 / np.sqrt((deg + 1).astype(np.float32))

    in_maps = []
    for c in range(NC):
        dis_own = np.ones(NPC, np.float32)
        mem = prep["core_nodes"][c]
        dis_own[:len(mem)] = dis_full[mem]
        in_maps.append(dict(
            xT=xT, degT=degT, dis_own=dis_own,
            W1=W1, W2=W2, Wc=Wc.reshape(OUT, 1),
            b1=b1, b2=b2, bc=np.full(128, float(bc.reshape(-1)[0]), np.float32),
            ident=np.eye(128, dtype=np.float32),
            idx=prep["idx_arrays"][c],
        ))

    trace = os.environ.get("GCN_TRACE", "0") == "1"
    if trace:
        _install_ntff_shim()
        import concourse.bass_utils as bu
        bu.upload_artifacts = lambda d: "local://" + str(d)
    res = run_bass_kernel_spmd(prog, in_maps, list(range(NC)), trace=trace)
    if trace:
        kernel.last_exec_time_ns = res.exec_time_ns

    out = np.zeros(N, np.float32)
    for c in range(NC):
        mem = prep["core_nodes"][c]
        out[mem] = res.results[c]["out"][:len(mem)]
    return out


kernel.last_exec_time_ns = None
